# revision 13
# baseline (speedup 1.0000x reference)
"""RON (recurrent oscillatory network) 8-core Trainium2 Bass kernel.

Shards the 64 modules across 8 NeuronCores (8 modules/core). Each core owns
x2h/h2h/bias/connection-matrix rows for its modules. Per time step the cores
exchange hy via an AllGather collective (HBM bounce); the local h2h matvec
overlaps the collective. x2h projections for all T steps are precomputed into
DRAM in one batched matmul phase.

Self-contained: the grading harness calls kernel(**inputs) with the full
(unsharded) inputs; sharding/gathering happens here.
"""

import os
import sys
import types

import numpy as np

# NTFF profile hook shim (the image's antenv lacks axon_hooks); harmless if
# tracing is never requested.
try:
    import antenv
    if 'antenv.axon_hooks' not in sys.modules:
        _m = types.ModuleType('antenv.axon_hooks')
        _h = [None]
        _m.set_axon_ntff_profile_hook = lambda h: _h.__setitem__(0, h)
        _m.get_axon_ntff_profile_hook = lambda: _h[0]
        sys.modules['antenv.axon_hooks'] = _m
        antenv.axon_hooks = _m
        try:
            from trn_agent_boot.trn_boot import _ntff_profile_via_ctypes
            hook = _ntff_profile_via_ctypes('/opt/axon/libaxon_pjrt.so')
            _m.set_axon_ntff_profile_hook(hook)
        except Exception:
            pass
except Exception:
    pass

import concourse.bass as bass
import concourse.bacc as bacc
import concourse.mybir as mybir
import concourse.tile as tile
from concourse import bass_utils

bass_utils.upload_artifacts = lambda d: d

DT = 0.01
N_CORES = 8
N_MOD, N_HID, N_INP = 64, 256, 128
MPC = N_MOD // N_CORES  # 8 modules per core
T_FULL = 2048

F32 = mybir.dt.float32
F32R = mybir.dt.float32r
AF = mybir.ActivationFunctionType


def _r(ap):
    """Matmul operand dtype view (f32r needs producer-side rounding on HW;
    keep plain f32 for now)."""
    return ap


def build_kernel(T=T_FULL):
    nc = bacc.Bacc("TRN2", target_bir_lowering=False, debug=False,
                   num_devices=N_CORES, detect_race_conditions=False)

    # ---- I/O ----
    x_in = nc.dram_tensor("x", [T, N_INP], F32, kind="ExternalInput")
    init_in = nc.dram_tensor("init", [MPC, 2, N_HID], F32, kind="ExternalInput")
    wm_in = nc.dram_tensor("wm", [N_HID, N_HID], F32, kind="ExternalInput")
    cm_in = nc.dram_tensor("cm", [MPC, N_MOD], F32, kind="ExternalInput")
    x2h_in = nc.dram_tensor("x2h", [MPC, N_HID, N_INP], F32, kind="ExternalInput")
    h2h_in = nc.dram_tensor("h2h", [MPC, N_HID, N_HID], F32, kind="ExternalInput")
    bias_in = nc.dram_tensor("bias", [MPC, N_HID], F32, kind="ExternalInput")
    ident_in = nc.dram_tensor("ident", [128, 128], F32, kind="ExternalInput")

    out_states = nc.dram_tensor("out_states", [T + 1, MPC, 2, N_HID], F32,
                                kind="ExternalOutput")
    out_fb = nc.dram_tensor("out_fb", [T, MPC, N_HID], F32, kind="ExternalOutput")

    NT = (T + 127) // 128  # t-chunks for the xproj precompute

    with tile.TileContext(nc) as tc:
        with tc.tile_pool(name="const", bufs=1) as cpool, \
             tc.tile_pool(name="dram", bufs=1, space="DRAM") as dpool:

            ident = cpool.tile([128, 128], F32, name="ident")
            nc.sync.dma_start(ident[:], ident_in[:])

            # ---- constants in SBUF ----
            bias_sb = cpool.tile([MPC, N_HID], F32, name="bias_sb")
            nc.sync.dma_start(bias_sb[:], bias_in[:])
            bias_r0 = cpool.tile([1, MPC * N_HID], F32, name="bias_r0")
            nc.sync.dma_start(bias_r0[:], bias_in[:].rearrange("n h -> (n h)")[None, :])

            cm_nat = cpool.tile([MPC, N_MOD], F32, name="cm_nat")
            nc.sync.dma_start(cm_nat[:], cm_in[:])
            cmT = cpool.tile([N_MOD, MPC], F32, name="cmT")

            wm_nat = cpool.tile([128, 2 * N_HID], F32, name="wm_nat")
            # rows g-half gh at cols [gh*256, gh*256+256)
            for gh in range(2):
                nc.sync.dma_start(wm_nat[:, gh * N_HID:(gh + 1) * N_HID],
                                  wm_in[gh * 128:(gh + 1) * 128, :])
            wmT = [cpool.tile([128, N_HID], F32, name=f"wmT{q}") for q in range(2)]

            # h2h natural & transposed
            h2h_nat = cpool.tile([128, MPC * 2 * N_HID], F32, name="h2h_nat")
            for n in range(MPC):
                for hh in range(2):
                    nc.sync.dma_start(
                        h2h_nat[:, n * 2 * N_HID + hh * N_HID:
                                n * 2 * N_HID + (hh + 1) * N_HID],
                        h2h_in[n, hh * 128:(hh + 1) * 128, :])
            h2hT = cpool.tile([128, MPC * 2 * N_HID], F32, name="h2hT")
            # h2hT layout: [o' , n*512 + q*256 + h] where o = q*128 + o'

            # x2h natural & transposed
            x2h_nat = cpool.tile([128, MPC * 2 * N_INP], F32, name="x2h_nat")
            for n in range(MPC):
                for hh in range(2):
                    nc.sync.dma_start(
                        x2h_nat[:, n * 2 * N_INP + hh * N_INP:
                                n * 2 * N_INP + (hh + 1) * N_INP],
                        x2h_in[n, hh * 128:(hh + 1) * 128, :])
            x2hT = cpool.tile([128, MPC * N_HID], F32, name="x2hT")
            # x2hT layout: [i, n*256 + h]

            # x transposed: [i, t]
            xT = cpool.tile([128, T], F32, name="xT")

            # states
            hy0 = cpool.tile([MPC, N_HID], F32, name="hy0")
            hz0 = cpool.tile([MPC, N_HID], F32, name="hz0")
            nc.sync.dma_start(hy0[:], init_in[:, 0, :])
            nc.sync.dma_start(hz0[:], init_in[:, 1, :])
            hyT0 = cpool.tile([128, 2 * MPC], F32, name="hyT0")
            hzT0 = cpool.tile([128, 2 * MPC], F32, name="hzT0")
            biasT = cpool.tile([128, 2 * MPC], F32, name="biasT")

            zero8 = cpool.tile([MPC, N_HID], F32, name="zero8")
            nc.vector.memset(zero8[:], 0.0)

            # out_states[0] = initial states
            st0 = cpool.tile([MPC, 2 * N_HID], F32, name="st0")
            nc.sync.dma_start(st0[:], init_in[:].rearrange("n s h -> n (s h)"))
            nc.sync.dma_start(
                out_states[0].rearrange("n s h -> n (s h)"), st0[:])

            # ---- init transposes (PE) ----
            with tc.tile_pool(name="ipsum", bufs=2, space="PSUM") as ipsum:
                def pe_t(dst_ap, src_ap, k):
                    """dst (SBUF) = src.T via PE transpose; src (k, m<=128)."""
                    pt = ipsum.tile([128, 128], F32, tag="pt", name="pt")
                    m = src_ap.shape[1]
                    nc.tensor.transpose(pt[:m, :k], src_ap, ident[:k, :k])
                    nc.vector.tensor_copy(dst_ap, pt[:m, :k])

                pe_t(cmT[:], cm_nat[:], MPC)
                for q in range(2):
                    pe_t(biasT[:, q * MPC:(q + 1) * MPC],
                         bias_sb[:, q * 128:(q + 1) * 128], MPC)
                for gh in range(2):
                    for oh in range(2):
                        pe_t(wmT[oh][:, gh * 128:(gh + 1) * 128],
                             wm_nat[:, gh * N_HID + oh * 128: gh * N_HID + (oh + 1) * 128],
                             128)
                for n in range(MPC):
                    for hh in range(2):
                        for oh in range(2):
                            # src rows h-half hh of h2h[n]: cols o-half oh
                            src = h2h_nat[:, n * 2 * N_HID + hh * N_HID + oh * 128:
                                          n * 2 * N_HID + hh * N_HID + (oh + 1) * 128]
                            dst = h2hT[:, n * 2 * N_HID + oh * N_HID + hh * 128:
                                       n * 2 * N_HID + oh * N_HID + (hh + 1) * 128]
                            pe_t(dst, src, 128)
                for n in range(MPC):
                    for hh in range(2):
                        src = x2h_nat[:, n * 2 * N_INP + hh * N_INP:
                                      n * 2 * N_INP + (hh + 1) * N_INP]
                        dst = x2hT[:, n * N_HID + hh * 128: n * N_HID + (hh + 1) * 128]
                        pe_t(dst, src, 128)
                # x: chunks (<=128t, 128i) -> xT[i, t]
                xnat = cpool.tile([128, N_INP], F32, name="xnat")
                for tc_i in range(NT):
                    tn = min(128, T - tc_i * 128)
                    nc.sync.dma_start(xnat[:tn, :],
                                      x_in[tc_i * 128: tc_i * 128 + tn, :])
                    pe_t(xT[:, tc_i * 128: tc_i * 128 + tn], xnat[:tn, :], tn)
                # hyT0 / hzT0
                for src0, dst0 in ((hy0, hyT0), (hz0, hzT0)):
                    for q in range(2):
                        pt = ipsum.tile([128, 128], F32, tag="pt", name="pt2")
                        nc.tensor.transpose(pt[:128, :MPC],
                                            src0[:, q * 128:(q + 1) * 128],
                                            ident[:MPC, :MPC])
                        nc.vector.tensor_copy(dst0[:, q * MPC:(q + 1) * MPC],
                                              pt[:128, :MPC])

            # ---- xproj precompute: xproj_dram[t, n, h] = x2h[n] @ x[t] + bias[n] ----
            xproj_dram = dpool.tile([T, MPC, N_HID], F32, name="xproj_dram")
            ones1 = cpool.tile([1, 128], F32, name="ones1")
            nc.vector.memset(ones1[:], 1.0)
            with tc.tile_pool(name="xpsum", bufs=4, space="PSUM") as xpsum, \
                 tc.tile_pool(name="xsb", bufs=4) as xsb:
                for n in range(MPC):
                    for tc_i in range(NT):
                        tn = min(128, T - tc_i * 128)
                        px = xpsum.tile([128, N_HID], F32, tag="px", name="px")
                        nc.tensor.matmul(
                            px[:tn, :], _r(xT[:, tc_i * 128: tc_i * 128 + tn]),
                            _r(x2hT[:, n * N_HID:(n + 1) * N_HID]),
                            start=True, stop=True)
                        xs = xsb.tile([128, N_HID], F32, tag="xs", name="xs")
                        nc.vector.tensor_copy(xs[:tn, :], px[:tn, :])
                        nc.sync.dma_start(
                            xproj_dram[tc_i * 128: tc_i * 128 + tn, n, :], xs[:tn, :])

            # ---- main recurrence ----
            with tc.tile_pool(name="mp", bufs=2, space="PSUM") as mpsum, \
                 tc.tile_pool(name="ms", bufs=3) as msb, \
                 tc.tile_pool(name="md", bufs=2, space="DRAM") as mdram:

                # T-layout state tiles: (128, 16), col = q*8 + j (q = o-half)
                hyT, hzT = hyT0, hzT0
                g_cur = None  # gathered outs for this step (None -> zeros)

                for t in range(T):
                    # prefetched xproj (+bias), natural (8, 256)
                    xp = msb.tile([MPC, N_HID], F32, tag="xp", name="xp")
                    nc.sync.dma_start(xp[:], xproj_dram[t])

                    # preT (128, 16): bias (start) + xp.T + h2h.hy [+ fb.T]
                    preT = mpsum.tile([128, 2 * MPC], F32, tag="pre", name="preT")
                    nc.tensor.matmul(preT[:], _r(ident[:]), _r(biasT[:]),
                                     start=True, stop=False, skip_group_check=True)
                    for q in range(2):
                        nc.tensor.matmul(preT[:, q * MPC:(q + 1) * MPC],
                                         xp[:, q * 128:(q + 1) * 128],
                                         ident[:MPC, :MPC], is_transpose=True,
                                         start=False, stop=False,
                                         skip_group_check=True)
                    # += h2h . hy   (stationary = h2hT block, moving = hyT col)
                    for j in range(MPC):
                        for q in range(2):      # o-half (contraction)
                            for hh in range(2):  # h-half (output partition)
                                last = (g_cur is None and j == MPC - 1
                                        and q == 1 and hh == 1)
                                nc.tensor.matmul(
                                    preT[:, hh * MPC + j: hh * MPC + j + 1],
                                    _r(h2hT[:, j * 2 * N_HID + q * N_HID + hh * 128:
                                            j * 2 * N_HID + q * N_HID + (hh + 1) * 128]),
                                    _r(hyT[:, q * MPC + j: q * MPC + j + 1]),
                                    start=False, stop=last,
                                    skip_group_check=True)

                    # feedback path (t>=1): fb = cm_own @ gathered @ wm.T
                    if g_cur is not None:
                        pz = mpsum.tile([MPC, N_HID], F32, tag="z", name="pz", bufs=1)
                        nc.tensor.matmul(pz[:], _r(cmT[:]), _r(g_cur[:]),
                                         start=True, stop=True)
                        z_sb = msb.tile([MPC, N_HID], F32, tag="zs", name="z_sb")
                        nc.vector.tensor_copy(z_sb[:], pz[:])
                        pzT = mpsum.tile([128, 2 * MPC], F32, tag="t16", name="pzT", bufs=1)
                        for q in range(2):
                            nc.tensor.transpose(
                                pzT[:, q * MPC:(q + 1) * MPC],
                                z_sb[:, q * 128:(q + 1) * 128],
                                ident[:MPC, :MPC])
                        zT = msb.tile([128, 2 * MPC], F32, tag="zT", name="zT")
                        nc.vector.tensor_copy(zT[:], pzT[:])
                        pfb = mpsum.tile([MPC, N_HID], F32, tag="fb", name="pfb")
                        for q in range(2):
                            nc.tensor.matmul(pfb[:],
                                             _r(zT[:, q * MPC:(q + 1) * MPC]),
                                             _r(wmT[q][:]),
                                             start=(q == 0), stop=(q == 1))
                        fb_sb = msb.tile([MPC, N_HID], F32, tag="fbs", name="fb_sb")
                        nc.vector.tensor_copy(fb_sb[:], pfb[:])
                        nc.sync.dma_start(out_fb[t], fb_sb[:])
                        # preT += fb.T (accumulating transposes)
                        for hh in range(2):
                            nc.tensor.matmul(
                                preT[:, hh * MPC:(hh + 1) * MPC],
                                fb_sb[:, hh * 128:(hh + 1) * 128],
                                ident[:MPC, :MPC],
                                is_transpose=True, start=False, stop=(hh == 1),
                                skip_group_check=True)
                    else:
                        nc.sync.dma_start(out_fb[t], zero8[:])

                    # tanh (T layout)
                    thT = msb.tile([128, 2 * MPC], F32, tag="th", name="thT")
                    nc.scalar.activation(thT[:], preT[:], AF.Tanh)

                    # state update (T layout):
                    #   e  = (1-DT)*hz - DT*hy          (early)
                    #   v  = hy + DT*e                  (early)
                    #   hy' = v + DT^2*tanh             (critical)
                    #   hz' = e + DT*tanh
                    e = msb.tile([128, 2 * MPC], F32, tag="e", name="e")
                    tmp = msb.tile([128, 2 * MPC], F32, tag="tmp", name="tmp")
                    nc.vector.tensor_scalar_mul(e[:], hzT[:], 1.0 - DT)
                    nc.vector.tensor_scalar_mul(tmp[:], hyT[:], DT)
                    nc.vector.tensor_sub(e[:], e[:], tmp[:])
                    v = msb.tile([128, 2 * MPC], F32, tag="v", name="v")
                    nc.vector.tensor_scalar_mul(v[:], e[:], DT)
                    nc.vector.tensor_add(v[:], v[:], hyT[:])

                    s2 = msb.tile([128, 2 * MPC], F32, tag="s2", name="s2")
                    nc.vector.tensor_scalar_mul(s2[:], thT[:], DT * DT)
                    hyT_n = msb.tile([128, 2 * MPC], F32, tag="hy", name="hyT_n")
                    nc.vector.tensor_add(hyT_n[:], v[:], s2[:])

                    s1 = msb.tile([128, 2 * MPC], F32, tag="s1", name="s1")
                    nc.vector.tensor_scalar_mul(s1[:], thT[:], DT)
                    hzT_n = msb.tile([128, 2 * MPC], F32, tag="hz", name="hzT_n")
                    nc.vector.tensor_add(hzT_n[:], e[:], s1[:])

                    # back to natural for outputs / exchange
                    pnat = mpsum.tile([MPC, 2 * N_HID], F32, tag="nat", name="pnat")
                    for q in range(2):
                        nc.tensor.transpose(pnat[:, q * 128:(q + 1) * 128],
                                            hyT_n[:, q * MPC:(q + 1) * MPC],
                                            ident[:128, :128])
                        nc.tensor.transpose(pnat[:, N_HID + q * 128:
                                                 N_HID + (q + 1) * 128],
                                            hzT_n[:, q * MPC:(q + 1) * MPC],
                                            ident[:128, :128])
                    hy_nat = msb.tile([MPC, N_HID], F32, tag="hyn", name="hy_nat")
                    hz_nat = msb.tile([MPC, N_HID], F32, tag="hzn", name="hz_nat")
                    nc.vector.tensor_copy(hy_nat[:], pnat[:, 0:N_HID])
                    nc.vector.tensor_copy(hz_nat[:], pnat[:, N_HID:2 * N_HID])

                    nc.sync.dma_start(out_states[t + 1, :, 0, :], hy_nat[:])
                    nc.sync.dma_start(out_states[t + 1, :, 1, :], hz_nat[:])

                    # exchange hy_{t+1} for step t+1's feedback
                    if t < T - 1:
                        bin_ = mdram.tile([MPC, N_HID], F32, tag="bin", name="bin_")
                        nc.sync.dma_start(bin_[:], hy_nat[:])
                        bout = mdram.tile([N_MOD, N_HID], F32, tag="bout", name="bout")
                        nc.gpsimd.collective_compute(
                            "AllGather", mybir.AluOpType.bypass,
                            replica_groups=[list(range(N_CORES))],
                            ins=[bin_[:].opt()], outs=[bout[:].opt()])
                        g_nxt = msb.tile([N_MOD, N_HID], F32, tag="g", name="g_nxt")
                        nc.sync.dma_start(g_nxt[:], bout[:])
                        g_cur = g_nxt

                    hyT, hzT = hyT_n, hzT_n

    nc.compile()
    return nc


_NC_CACHE = {}


def _get_nc(T):
    if T not in _NC_CACHE:
        _NC_CACHE[T] = build_kernel(T)
    return _NC_CACHE[T]


def make_in_maps(x, initial_states, wm_weight, connection_matrix, x2h, h2h, bias):
    ident = np.eye(128, dtype=np.float32)
    in_maps = []
    for c in range(N_CORES):
        s = slice(c * MPC, (c + 1) * MPC)
        in_maps.append({
            "x": np.ascontiguousarray(x, dtype=np.float32),
            "init": np.ascontiguousarray(initial_states[s], dtype=np.float32),
            "wm": np.ascontiguousarray(wm_weight, dtype=np.float32),
            "cm": np.ascontiguousarray(connection_matrix[s], dtype=np.float32),
            "x2h": np.ascontiguousarray(x2h[s], dtype=np.float32),
            "h2h": np.ascontiguousarray(h2h[s], dtype=np.float32),
            "bias": np.ascontiguousarray(bias[s], dtype=np.float32),
            "ident": ident,
        })
    return in_maps


def kernel(x, initial_states, wm_weight, connection_matrix, x2h, h2h, bias):
    x = np.asarray(x)
    T = x.shape[0]
    nc = _get_nc(T)
    in_maps = make_in_maps(x, initial_states, wm_weight, connection_matrix,
                           x2h, h2h, bias)
    res = bass_utils.run_bass_kernel_spmd(
        nc, in_maps, core_ids=list(range(N_CORES)))
    states = np.concatenate([res.results[c]["out_states"] for c in range(N_CORES)],
                            axis=1)
    fb = np.concatenate([res.results[c]["out_fb"] for c in range(N_CORES)], axis=1)
    return states.astype(np.float32), fb.astype(np.float32)


# revision 14
# speedup vs baseline: 1.0647x; 1.0647x over previous
"""RON (recurrent oscillatory network) 8-core Trainium2 Bass kernel.

Shards the 64 modules across 8 NeuronCores (8 modules/core). Each core owns
x2h/h2h/bias/connection-matrix rows for its modules. Per time step the cores
exchange hy via an AllGather collective (HBM bounce); the local h2h matvec
overlaps the collective. x2h projections for all T steps are precomputed into
DRAM in one batched matmul phase.

Self-contained: the grading harness calls kernel(**inputs) with the full
(unsharded) inputs; sharding/gathering happens here.
"""

import os
import sys
import types

import numpy as np

# NTFF profile hook shim (the image's antenv lacks axon_hooks); harmless if
# tracing is never requested.
try:
    import antenv
    if 'antenv.axon_hooks' not in sys.modules:
        _m = types.ModuleType('antenv.axon_hooks')
        _h = [None]
        _m.set_axon_ntff_profile_hook = lambda h: _h.__setitem__(0, h)
        _m.get_axon_ntff_profile_hook = lambda: _h[0]
        sys.modules['antenv.axon_hooks'] = _m
        antenv.axon_hooks = _m
        try:
            from trn_agent_boot.trn_boot import _ntff_profile_via_ctypes
            hook = _ntff_profile_via_ctypes('/opt/axon/libaxon_pjrt.so')
            _m.set_axon_ntff_profile_hook(hook)
        except Exception:
            pass
except Exception:
    pass

import concourse.bass as bass
import concourse.bacc as bacc
import concourse.mybir as mybir
import concourse.tile as tile
from concourse import bass_utils

bass_utils.upload_artifacts = lambda d: d

DT = 0.01
N_CORES = 8
N_MOD, N_HID, N_INP = 64, 256, 128
MPC = N_MOD // N_CORES  # 8 modules per core
T_FULL = 2048

F32 = mybir.dt.float32
F32R = mybir.dt.float32r
AF = mybir.ActivationFunctionType


def _r(ap):
    """Matmul operand dtype view (f32r needs producer-side rounding on HW;
    keep plain f32 for now)."""
    return ap


def build_kernel(T=T_FULL):
    nc = bacc.Bacc("TRN2", target_bir_lowering=False, debug=False,
                   num_devices=N_CORES, detect_race_conditions=False)

    # ---- I/O ----
    x_in = nc.dram_tensor("x", [T, N_INP], F32, kind="ExternalInput")
    init_in = nc.dram_tensor("init", [MPC, 2, N_HID], F32, kind="ExternalInput")
    wm_in = nc.dram_tensor("wm", [N_HID, N_HID], F32, kind="ExternalInput")
    cm_in = nc.dram_tensor("cm", [MPC, N_MOD], F32, kind="ExternalInput")
    x2h_in = nc.dram_tensor("x2h", [MPC, N_HID, N_INP], F32, kind="ExternalInput")
    h2h_in = nc.dram_tensor("h2h", [MPC, N_HID, N_HID], F32, kind="ExternalInput")
    bias_in = nc.dram_tensor("bias", [MPC, N_HID], F32, kind="ExternalInput")
    ident_in = nc.dram_tensor("ident", [128, 128], F32, kind="ExternalInput")

    out_states = nc.dram_tensor("out_states", [T + 1, MPC, 2, N_HID], F32,
                                kind="ExternalOutput")
    out_fb = nc.dram_tensor("out_fb", [T, MPC, N_HID], F32, kind="ExternalOutput")

    NT = (T + 127) // 128  # t-chunks for the xproj precompute

    with tile.TileContext(nc) as tc:
        with tc.tile_pool(name="const", bufs=1) as cpool, \
             tc.tile_pool(name="dram", bufs=1, space="DRAM") as dpool:

            ident = cpool.tile([128, 128], F32, name="ident")
            nc.sync.dma_start(ident[:], ident_in[:])

            # ---- constants in SBUF ----
            bias_sb = cpool.tile([MPC, N_HID], F32, name="bias_sb")
            nc.sync.dma_start(bias_sb[:], bias_in[:])
            bias_r0 = cpool.tile([1, MPC * N_HID], F32, name="bias_r0")
            nc.sync.dma_start(bias_r0[:], bias_in[:].rearrange("n h -> (n h)")[None, :])

            cm_nat = cpool.tile([MPC, N_MOD], F32, name="cm_nat")
            nc.sync.dma_start(cm_nat[:], cm_in[:])
            cmT = cpool.tile([N_MOD, MPC], F32, name="cmT")

            wm_nat = cpool.tile([128, 2 * N_HID], F32, name="wm_nat")
            # rows g-half gh at cols [gh*256, gh*256+256)
            for gh in range(2):
                nc.sync.dma_start(wm_nat[:, gh * N_HID:(gh + 1) * N_HID],
                                  wm_in[gh * 128:(gh + 1) * 128, :])
            wmT = [cpool.tile([128, N_HID], mybir.dt.bfloat16, name=f"wmT{q}") for q in range(2)]

            # h2h natural & transposed
            h2h_nat = cpool.tile([128, MPC * 2 * N_HID], F32, name="h2h_nat")
            for n in range(MPC):
                for hh in range(2):
                    nc.sync.dma_start(
                        h2h_nat[:, n * 2 * N_HID + hh * N_HID:
                                n * 2 * N_HID + (hh + 1) * N_HID],
                        h2h_in[n, hh * 128:(hh + 1) * 128, :])
            h2hT = cpool.tile([128, MPC * 2 * N_HID], mybir.dt.bfloat16, name="h2hT")
            # h2hT layout: [o' , n*512 + q*256 + h] where o = q*128 + o'

            # x2h natural & transposed
            x2h_nat = cpool.tile([128, MPC * 2 * N_INP], F32, name="x2h_nat")
            for n in range(MPC):
                for hh in range(2):
                    nc.sync.dma_start(
                        x2h_nat[:, n * 2 * N_INP + hh * N_INP:
                                n * 2 * N_INP + (hh + 1) * N_INP],
                        x2h_in[n, hh * 128:(hh + 1) * 128, :])
            x2hT = cpool.tile([128, MPC * N_HID], F32, name="x2hT")
            # x2hT layout: [i, n*256 + h]

            # x transposed: [i, t]
            xT = cpool.tile([128, T], F32, name="xT")

            # states
            hy0 = cpool.tile([MPC, N_HID], F32, name="hy0")
            hz0 = cpool.tile([MPC, N_HID], F32, name="hz0")
            nc.sync.dma_start(hy0[:], init_in[:, 0, :])
            nc.sync.dma_start(hz0[:], init_in[:, 1, :])
            hyT0 = cpool.tile([128, 2 * MPC], F32, name="hyT0")
            hzT0 = cpool.tile([128, 2 * MPC], F32, name="hzT0")
            biasT = cpool.tile([128, 2 * MPC], F32, name="biasT")

            zero8 = cpool.tile([MPC, N_HID], F32, name="zero8")
            nc.vector.memset(zero8[:], 0.0)

            # out_states[0] = initial states
            st0 = cpool.tile([MPC, 2 * N_HID], F32, name="st0")
            nc.sync.dma_start(st0[:], init_in[:].rearrange("n s h -> n (s h)"))
            nc.sync.dma_start(
                out_states[0].rearrange("n s h -> n (s h)"), st0[:])

            # ---- init transposes (PE) ----
            with tc.tile_pool(name="ipsum", bufs=2, space="PSUM") as ipsum:
                def pe_t(dst_ap, src_ap, k):
                    """dst (SBUF) = src.T via PE transpose; src (k, m<=128)."""
                    pt = ipsum.tile([128, 128], F32, tag="pt", name="pt")
                    m = src_ap.shape[1]
                    nc.tensor.transpose(pt[:m, :k], src_ap, ident[:k, :k])
                    nc.vector.tensor_copy(dst_ap, pt[:m, :k])

                pe_t(cmT[:], cm_nat[:], MPC)
                for q in range(2):
                    pe_t(biasT[:, q * MPC:(q + 1) * MPC],
                         bias_sb[:, q * 128:(q + 1) * 128], MPC)
                for gh in range(2):
                    for oh in range(2):
                        pe_t(wmT[oh][:, gh * 128:(gh + 1) * 128],
                             wm_nat[:, gh * N_HID + oh * 128: gh * N_HID + (oh + 1) * 128],
                             128)
                for n in range(MPC):
                    for hh in range(2):
                        for oh in range(2):
                            # src rows h-half hh of h2h[n]: cols o-half oh
                            src = h2h_nat[:, n * 2 * N_HID + hh * N_HID + oh * 128:
                                          n * 2 * N_HID + hh * N_HID + (oh + 1) * 128]
                            dst = h2hT[:, n * 2 * N_HID + oh * N_HID + hh * 128:
                                       n * 2 * N_HID + oh * N_HID + (hh + 1) * 128]
                            pe_t(dst, src, 128)
                for n in range(MPC):
                    for hh in range(2):
                        src = x2h_nat[:, n * 2 * N_INP + hh * N_INP:
                                      n * 2 * N_INP + (hh + 1) * N_INP]
                        dst = x2hT[:, n * N_HID + hh * 128: n * N_HID + (hh + 1) * 128]
                        pe_t(dst, src, 128)
                # x: chunks (<=128t, 128i) -> xT[i, t]
                xnat = cpool.tile([128, N_INP], F32, name="xnat")
                for tc_i in range(NT):
                    tn = min(128, T - tc_i * 128)
                    nc.sync.dma_start(xnat[:tn, :],
                                      x_in[tc_i * 128: tc_i * 128 + tn, :])
                    pe_t(xT[:, tc_i * 128: tc_i * 128 + tn], xnat[:tn, :], tn)
                # hyT0 / hzT0
                for src0, dst0 in ((hy0, hyT0), (hz0, hzT0)):
                    for q in range(2):
                        pt = ipsum.tile([128, 128], F32, tag="pt", name="pt2")
                        nc.tensor.transpose(pt[:128, :MPC],
                                            src0[:, q * 128:(q + 1) * 128],
                                            ident[:MPC, :MPC])
                        nc.vector.tensor_copy(dst0[:, q * MPC:(q + 1) * MPC],
                                              pt[:128, :MPC])

            # ---- xproj precompute: xproj_dram[t, n, h] = x2h[n] @ x[t] + bias[n] ----
            xproj_dram = dpool.tile([T, MPC, N_HID], F32, name="xproj_dram")
            ones1 = cpool.tile([1, 128], F32, name="ones1")
            nc.vector.memset(ones1[:], 1.0)
            with tc.tile_pool(name="xpsum", bufs=4, space="PSUM") as xpsum, \
                 tc.tile_pool(name="xsb", bufs=4) as xsb:
                for n in range(MPC):
                    for tc_i in range(NT):
                        tn = min(128, T - tc_i * 128)
                        px = xpsum.tile([128, N_HID], F32, tag="px", name="px")
                        nc.tensor.matmul(
                            px[:tn, :], _r(xT[:, tc_i * 128: tc_i * 128 + tn]),
                            _r(x2hT[:, n * N_HID:(n + 1) * N_HID]),
                            start=True, stop=True)
                        xs = xsb.tile([128, N_HID], F32, tag="xs", name="xs")
                        nc.vector.tensor_copy(xs[:tn, :], px[:tn, :])
                        nc.sync.dma_start(
                            xproj_dram[tc_i * 128: tc_i * 128 + tn, n, :], xs[:tn, :])

            # ---- main recurrence ----
            with tc.tile_pool(name="mp", bufs=2, space="PSUM") as mpsum, \
                 tc.tile_pool(name="ms", bufs=3) as msb, \
                 tc.tile_pool(name="md", bufs=2, space="DRAM") as mdram:

                # T-layout state tiles: (128, 16), col = q*8 + j (q = o-half)
                hyT, hzT = hyT0, hzT0
                hyT_bf0 = cpool.tile([128, 2 * MPC], mybir.dt.bfloat16, name="hyT_bf0")
                nc.vector.tensor_copy(hyT_bf0[:], hyT0[:])
                hyT_bf = hyT_bf0
                g_cur = None  # gathered outs for this step (None -> zeros)

                for t in range(T):
                    # prefetched xproj (+bias), natural (8, 256)
                    xp = msb.tile([MPC, N_HID], F32, tag="xp", name="xp")
                    nc.sync.dma_start(xp[:], xproj_dram[t])

                    # preT (128, 16): bias (start) + xp.T + h2h.hy [+ fb.T]
                    preT = mpsum.tile([128, 2 * MPC], F32, tag="pre", name="preT")
                    nc.tensor.matmul(preT[:], _r(ident[:]), _r(biasT[:]),
                                     start=True, stop=False, skip_group_check=True)
                    for q in range(2):
                        nc.tensor.matmul(preT[:, q * MPC:(q + 1) * MPC],
                                         xp[:, q * 128:(q + 1) * 128],
                                         ident[:MPC, :MPC], is_transpose=True,
                                         start=False, stop=False,
                                         skip_group_check=True)
                    # += h2h . hy   (stationary = h2hT block, moving = hyT col)
                    for j in range(MPC):
                        for q in range(2):      # o-half (contraction)
                            for hh in range(2):  # h-half (output partition)
                                last = (g_cur is None and j == MPC - 1
                                        and q == 1 and hh == 1)
                                nc.tensor.matmul(
                                    preT[:, hh * MPC + j: hh * MPC + j + 1],
                                    _r(h2hT[:, j * 2 * N_HID + q * N_HID + hh * 128:
                                            j * 2 * N_HID + q * N_HID + (hh + 1) * 128]),
                                    hyT_bf[:, q * MPC + j: q * MPC + j + 1],
                                    start=False, stop=last,
                                    skip_group_check=True)

                    # feedback path (t>=1): fb = cm_own @ gathered @ wm.T
                    if g_cur is not None:
                        pz = mpsum.tile([MPC, N_HID], F32, tag="z", name="pz", bufs=1)
                        nc.tensor.matmul(pz[:], _r(cmT[:]), _r(g_cur[:]),
                                         start=True, stop=True)
                        z_sb = msb.tile([MPC, N_HID], F32, tag="zs", name="z_sb")
                        nc.vector.tensor_copy(z_sb[:], pz[:])
                        pzT = mpsum.tile([128, 2 * MPC], F32, tag="t16", name="pzT", bufs=1)
                        for q in range(2):
                            nc.tensor.transpose(
                                pzT[:, q * MPC:(q + 1) * MPC],
                                z_sb[:, q * 128:(q + 1) * 128],
                                ident[:MPC, :MPC])
                        zT = msb.tile([128, 2 * MPC], mybir.dt.bfloat16, tag="zT", name="zT")
                        nc.vector.tensor_copy(zT[:], pzT[:])
                        pfb = mpsum.tile([MPC, N_HID], F32, tag="fb", name="pfb")
                        for q in range(2):
                            nc.tensor.matmul(pfb[:],
                                             _r(zT[:, q * MPC:(q + 1) * MPC]),
                                             _r(wmT[q][:]),
                                             start=(q == 0), stop=(q == 1))
                        fb_sb = msb.tile([MPC, N_HID], F32, tag="fbs", name="fb_sb")
                        nc.vector.tensor_copy(fb_sb[:], pfb[:])
                        nc.sync.dma_start(out_fb[t], fb_sb[:])
                        # preT += fb.T (accumulating transposes)
                        for hh in range(2):
                            nc.tensor.matmul(
                                preT[:, hh * MPC:(hh + 1) * MPC],
                                fb_sb[:, hh * 128:(hh + 1) * 128],
                                ident[:MPC, :MPC],
                                is_transpose=True, start=False, stop=(hh == 1),
                                skip_group_check=True)
                    else:
                        nc.sync.dma_start(out_fb[t], zero8[:])

                    # tanh (T layout)
                    thT = msb.tile([128, 2 * MPC], F32, tag="th", name="thT")
                    nc.scalar.activation(thT[:], preT[:], AF.Tanh)

                    # state update (T layout):
                    #   e  = (1-DT)*hz - DT*hy          (early)
                    #   v  = hy + DT*e                  (early)
                    #   hy' = v + DT^2*tanh             (critical)
                    #   hz' = e + DT*tanh
                    e = msb.tile([128, 2 * MPC], F32, tag="e", name="e")
                    tmp = msb.tile([128, 2 * MPC], F32, tag="tmp", name="tmp")
                    nc.vector.tensor_scalar_mul(e[:], hzT[:], 1.0 - DT)
                    nc.vector.tensor_scalar_mul(tmp[:], hyT[:], DT)
                    nc.vector.tensor_sub(e[:], e[:], tmp[:])
                    v = msb.tile([128, 2 * MPC], F32, tag="v", name="v")
                    nc.vector.tensor_scalar_mul(v[:], e[:], DT)
                    nc.vector.tensor_add(v[:], v[:], hyT[:])

                    s2 = msb.tile([128, 2 * MPC], F32, tag="s2", name="s2")
                    nc.vector.tensor_scalar_mul(s2[:], thT[:], DT * DT)
                    hyT_n = msb.tile([128, 2 * MPC], F32, tag="hy", name="hyT_n")
                    nc.vector.tensor_add(hyT_n[:], v[:], s2[:])

                    s1 = msb.tile([128, 2 * MPC], F32, tag="s1", name="s1")
                    nc.vector.tensor_scalar_mul(s1[:], thT[:], DT)
                    hzT_n = msb.tile([128, 2 * MPC], F32, tag="hz", name="hzT_n")
                    nc.vector.tensor_add(hzT_n[:], e[:], s1[:])

                    # back to natural for outputs / exchange
                    pnat = mpsum.tile([MPC, 2 * N_HID], F32, tag="nat", name="pnat")
                    for q in range(2):
                        nc.tensor.transpose(pnat[:, q * 128:(q + 1) * 128],
                                            hyT_n[:, q * MPC:(q + 1) * MPC],
                                            ident[:128, :128])
                        nc.tensor.transpose(pnat[:, N_HID + q * 128:
                                                 N_HID + (q + 1) * 128],
                                            hzT_n[:, q * MPC:(q + 1) * MPC],
                                            ident[:128, :128])
                    hy_nat = msb.tile([MPC, N_HID], F32, tag="hyn", name="hy_nat")
                    hz_nat = msb.tile([MPC, N_HID], F32, tag="hzn", name="hz_nat")
                    nc.vector.tensor_copy(hy_nat[:], pnat[:, 0:N_HID])
                    nc.vector.tensor_copy(hz_nat[:], pnat[:, N_HID:2 * N_HID])

                    nc.sync.dma_start(out_states[t + 1, :, 0, :], hy_nat[:])
                    nc.sync.dma_start(out_states[t + 1, :, 1, :], hz_nat[:])

                    # exchange hy_{t+1} for step t+1's feedback
                    if t < T - 1:
                        bin_ = mdram.tile([MPC, N_HID], F32, tag="bin", name="bin_")
                        nc.sync.dma_start(bin_[:], hy_nat[:])
                        bout = mdram.tile([N_MOD, N_HID], F32, tag="bout", name="bout")
                        nc.gpsimd.collective_compute(
                            "AllGather", mybir.AluOpType.bypass,
                            replica_groups=[list(range(N_CORES))],
                            ins=[bin_[:].opt()], outs=[bout[:].opt()])
                        g_nxt = msb.tile([N_MOD, N_HID], F32, tag="g", name="g_nxt")
                        nc.sync.dma_start(g_nxt[:], bout[:])
                        g_cur = g_nxt

                    hyT_bf_n = msb.tile([128, 2 * MPC], mybir.dt.bfloat16,
                                        tag="hybf", name="hyT_bf_n")
                    nc.vector.tensor_copy(hyT_bf_n[:], hyT_n[:])
                    hyT_bf = hyT_bf_n
                    hyT, hzT = hyT_n, hzT_n

    nc.compile()
    return nc


_NC_CACHE = {}


def _get_nc(T):
    if T not in _NC_CACHE:
        _NC_CACHE[T] = build_kernel(T)
    return _NC_CACHE[T]


def make_in_maps(x, initial_states, wm_weight, connection_matrix, x2h, h2h, bias):
    ident = np.eye(128, dtype=np.float32)
    in_maps = []
    for c in range(N_CORES):
        s = slice(c * MPC, (c + 1) * MPC)
        in_maps.append({
            "x": np.ascontiguousarray(x, dtype=np.float32),
            "init": np.ascontiguousarray(initial_states[s], dtype=np.float32),
            "wm": np.ascontiguousarray(wm_weight, dtype=np.float32),
            "cm": np.ascontiguousarray(connection_matrix[s], dtype=np.float32),
            "x2h": np.ascontiguousarray(x2h[s], dtype=np.float32),
            "h2h": np.ascontiguousarray(h2h[s], dtype=np.float32),
            "bias": np.ascontiguousarray(bias[s], dtype=np.float32),
            "ident": ident,
        })
    return in_maps


def kernel(x, initial_states, wm_weight, connection_matrix, x2h, h2h, bias):
    x = np.asarray(x)
    T = x.shape[0]
    nc = _get_nc(T)
    in_maps = make_in_maps(x, initial_states, wm_weight, connection_matrix,
                           x2h, h2h, bias)
    res = bass_utils.run_bass_kernel_spmd(
        nc, in_maps, core_ids=list(range(N_CORES)))
    states = np.concatenate([res.results[c]["out_states"] for c in range(N_CORES)],
                            axis=1)
    fb = np.concatenate([res.results[c]["out_fb"] for c in range(N_CORES)], axis=1)
    return states.astype(np.float32), fb.astype(np.float32)
